# revision 23
# baseline (speedup 1.0000x reference)
"""Trainium2 Bass kernel for AngularMinPooling.

out[v, r] = inputs[v, r, argmin_j ||inputs[v, j, :]||_2]
Input (500000, 8, 64) f32 -> Output (500000, 8) f32.
Vertices are sharded across 8 NeuronCores; no cross-core communication.

HBM-bound (128 MB read per core). DMA tiles are 4 MB (VPP=16 vertices per
partition -> 32 KB contiguous per partition line) to amortize per-packet
SDMA dead time. Compute is chunked at 8 vertices/partition and split so no
engine exceeds the DMA floor:
  ACT    squares a chunk into sqd
  GpSimd folds sqd[...,0:32] + sqd[...,32:64] (halves the DVE reduce)
  DVE    reduces the fold to squared norms, min-reduces, builds the is_le
         one-hot, and does the one-hot weighted gather over the first 8
         feature columns (exact vs argmin except on bitwise-equal ties).
Output is staged in SBUF for the whole shard and written once,
partition-major (the host undoes the permutation).
"""

import os
import sys

import numpy as np

for _p in ("/opt/trn_rl_repo",):
    if os.path.isdir(_p) and _p not in sys.path:
        sys.path.insert(0, _p)

import concourse.bacc as bacc
import concourse.bass as bass
import concourse.tile as tile
from concourse import mybir
from concourse.bass_utils import run_bass_kernel_spmd


def _ensure_ntff_hook():
    """Install the axon NTFF profile hook if the image's antenv lacks it."""
    import types

    try:
        from antenv.axon_hooks import get_axon_ntff_profile_hook  # noqa: F401

        return
    except ImportError:
        pass
    try:
        import antenv
        from trn_agent_boot.trn_boot import _ntff_profile_via_ctypes

        mod = types.ModuleType("antenv.axon_hooks")
        _state = {"hook": None}
        mod.set_axon_ntff_profile_hook = lambda h: _state.__setitem__("hook", h)
        mod.get_axon_ntff_profile_hook = lambda: _state["hook"]
        sys.modules["antenv.axon_hooks"] = mod
        antenv.axon_hooks = mod
        so_path = "/opt/axon/libaxon_pjrt.so"
        if os.path.exists(so_path):
            mod.set_axon_ntff_profile_hook(_ntff_profile_via_ctypes(so_path))
    except Exception:
        pass


_ensure_ntff_hook()

N_VERTICES = 500_000
R = 8
F = 64
FH = F // 2  # folded feature width
N_CORES = 8
N_SHARD = N_VERTICES // N_CORES  # 62500 vertices per core
P = 128  # SBUF partitions
VPP = 16  # vertices per partition per DMA tile
CV = 8  # vertices per partition per compute chunk
TILE_V = P * VPP  # 2048 vertices per DMA tile
N_FULL = N_SHARD // TILE_V  # 30 full tiles (61440 vertices)
REM = N_SHARD - N_FULL * TILE_V  # 1060
N_MID = REM // (P * CV)  # 1 extra VPP=8 tile (1024 vertices)
TAIL = REM - N_MID * P * CV  # 36 leftover vertices
N_SLOTS = N_FULL * VPP + N_MID * CV  # 488 staged vertex slots per partition
KF = int(os.environ.get("AMP_KF", "0"))  # features folded in-place by GpSimd
SKEW = int(os.environ.get("AMP_SKEW", "0"))  # chunks of deferred gather emission

_DT = mybir.dt.float32
_AX = mybir.AxisListType
_OP = mybir.AluOpType


def _build_nc():
    nc = bacc.Bacc(
        "TRN2",
        target_bir_lowering=False,
        debug=False,
        enable_asserts=False,
        num_devices=N_CORES,
    )
    x = nc.dram_tensor("inputs", [N_SHARD, R, F], _DT, kind="ExternalInput")
    # Partition-major staged output; the host undoes the permutation.
    raw = nc.dram_tensor("raw", [P, N_SLOTS, R], _DT, kind="ExternalOutput")
    traw = nc.dram_tensor("traw", [TAIL, R], _DT, kind="ExternalOutput")
    xa = x.ap()

    with tile.TileContext(nc) as tc:
        with (
            tc.tile_pool(name="xin", bufs=4) as xin_pool,
            tc.tile_pool(name="sqd", bufs=2) as sqd_pool,
            tc.tile_pool(name="work", bufs=4) as work_pool,
            tc.tile_pool(name="stage", bufs=1) as stage_pool,
        ):
            stage = stage_pool.tile([P, N_SLOTS, R], _DT)

            # Software-pipelined emission: chunk c's front phase (square,
            # GP kfold, DVE reduce+min) is emitted immediately; its gather
            # phase (DVE is_le, GP mult+folds) is deferred until the next
            # chunk's front phase is in the queues. This keeps GpSimd's FIFO
            # free of head-of-line blocking: kfold(c+1) sits BEFORE
            # gather(c), so it never waits behind DVE's chunk-c tail.
            pending = []

            def front_chunk(xt_c, pc, vpp, ot_dst):
                """xt_c: [pc, vpp, R, F] slice of a loaded tile."""
                sqd = sqd_pool.tile([P, CV, R, F], _DT, tag="sqd")
                nc.scalar.square(sqd[:pc, :vpp], xt_c)

                # GpSimd folds the last KF squared features onto the previous
                # KF in place, so DVE only reduces F - KF features.
                if KF:
                    nc.gpsimd.tensor_tensor(
                        out=sqd[:pc, :vpp, :, F - 2 * KF : F - KF],
                        in0=sqd[:pc, :vpp, :, F - 2 * KF : F - KF],
                        in1=sqd[:pc, :vpp, :, F - KF : F],
                        op=_OP.add,
                    )
                sq = work_pool.tile([P, CV, R], _DT, tag="sq")
                nc.vector.tensor_reduce(
                    out=sq[:pc, :vpp],
                    in_=sqd[:pc, :vpp, :, 0 : F - KF],
                    axis=_AX.X,
                    op=_OP.add,
                )
                m = work_pool.tile([P, CV], _DT, tag="m")
                nc.vector.tensor_reduce(
                    out=m[:pc, :vpp], in_=sq[:pc, :vpp], axis=_AX.X, op=_OP.min
                )
                pending.append((xt_c, pc, vpp, sq, m, ot_dst))

            def gather_chunk(xt_c, pc, vpp, sq, m, ot_dst):
                # One-hot at the min norm (multi-hot only on bitwise-equal
                # ties, which have ~0 probability for random f32 sums).
                sel = work_pool.tile([P, CV, R], _DT, tag="sel")
                nc.vector.tensor_tensor(
                    out=sel[:pc, :vpp],
                    in0=sq[:pc, :vpp],
                    in1=m[:pc, :vpp, None].broadcast_to([pc, vpp, R]),
                    op=_OP.is_le,
                )
                # Gather via one-hot weighted sum over the first R feature
                # columns (argmin index is always < R); the sum over j is a
                # 3-level pairwise fold, all on GpSimd.
                g = work_pool.tile([P, CV, R, R], _DT, tag="g")
                nc.gpsimd.tensor_tensor(
                    out=g[:pc, :vpp],
                    in0=xt_c[:, :, :, 0:R],
                    in1=sel[:pc, :vpp, None, :].broadcast_to([pc, vpp, R, R]),
                    op=_OP.mult,
                )
                g2 = work_pool.tile([P, CV, R, R // 2], _DT, tag="g2")
                nc.gpsimd.tensor_tensor(
                    out=g2[:pc, :vpp],
                    in0=g[:pc, :vpp, :, 0 : R // 2],
                    in1=g[:pc, :vpp, :, R // 2 : R],
                    op=_OP.add,
                )
                g3 = work_pool.tile([P, CV, R, R // 4], _DT, tag="g3")
                nc.gpsimd.tensor_tensor(
                    out=g3[:pc, :vpp],
                    in0=g2[:pc, :vpp, :, 0 : R // 4],
                    in1=g2[:pc, :vpp, :, R // 4 : R // 2],
                    op=_OP.add,
                )
                nc.gpsimd.tensor_tensor(
                    out=ot_dst[:, :, :, None],
                    in0=g3[:pc, :vpp, :, 0:1],
                    in1=g3[:pc, :vpp, :, 1:2],
                    op=_OP.add,
                )

            def do_chunk(xt_c, pc, vpp, ot_dst, skew=SKEW):
                front_chunk(xt_c, pc, vpp, ot_dst)
                while len(pending) > skew:
                    gather_chunk(*pending.pop(0))

            def drain_chunks():
                while pending:
                    gather_chunk(*pending.pop(0))

            # Flush the staged output in slices as tiles complete so only a
            # small write remains after the last compute. (Keyed later when
            # skewed gather emission delays slice coverage.)
            _s = 1 if SKEW else 0
            flush_after = {
                7 + _s: (0, 128),
                15 + _s: (128, 256),
                23 + _s: (256, 384),
            }
            for t in range(N_FULL):
                v0 = t * TILE_V
                xt = xin_pool.tile([P, VPP, R, F], _DT, tag="xt")
                src = xa[v0 : v0 + TILE_V].rearrange("(p v) r f -> p v r f", p=P)
                dma_eng = nc.sync if t % 2 == 0 else nc.scalar
                dma_eng.dma_start(out=xt[:], in_=src)
                for c in range(VPP // CV):
                    s0 = t * VPP + c * CV
                    do_chunk(
                        xt[:, c * CV : (c + 1) * CV], P, CV,
                        stage[:, s0 : s0 + CV],
                    )
                if t in flush_after:
                    lo, hi = flush_after[t]
                    nc.gpsimd.dma_start(
                        out=raw.ap()[:, lo:hi], in_=stage[:, lo:hi]
                    )

            if N_MID:
                v0 = N_FULL * TILE_V
                xt = xin_pool.tile([P, VPP, R, F], _DT, tag="xt")
                src = xa[v0 : v0 + P * CV].rearrange("(p v) r f -> p v r f", p=P)
                nc.sync.dma_start(out=xt[:, :CV], in_=src)
                s0 = N_FULL * VPP
                do_chunk(xt[:, :CV], P, CV, stage[:, s0 : s0 + CV])

            if TAIL:
                v0 = N_FULL * TILE_V + N_MID * P * CV
                xt = xin_pool.tile([P, VPP, R, F], _DT, tag="xt")
                src = xa[v0 : v0 + TAIL].rearrange("(p v) r f -> p v r f", p=TAIL)
                nc.sync.dma_start(out=xt[:TAIL, :1], in_=src)
                ot_tail = work_pool.tile([P, 1, R], _DT, tag="ot_tail")
                do_chunk(xt[:TAIL, :1], TAIL, 1, ot_tail[:TAIL, :1])

            drain_chunks()
            nc.gpsimd.dma_start(out=raw.ap()[:, 384:], in_=stage[:, 384:])
            if TAIL:
                nc.gpsimd.dma_start(out=traw.ap(), in_=ot_tail[:TAIL, :1])
    nc.finalize()
    return nc


_NC_CACHE = None


def _get_nc():
    global _NC_CACHE
    if _NC_CACHE is None:
        _NC_CACHE = _build_nc()
    return _NC_CACHE


def _decode_raw(raw_arr: np.ndarray, traw_arr: np.ndarray) -> np.ndarray:
    """Map staged [P, N_SLOTS, R] output back to vertex order."""
    raw_arr = np.asarray(raw_arr).astype(np.float32)
    full = (
        raw_arr[:, : N_FULL * VPP]
        .reshape(P, N_FULL, VPP, R)
        .transpose(1, 0, 2, 3)
        .reshape(N_FULL * TILE_V, R)
    )
    parts = [full]
    if N_MID:
        mid = raw_arr[:, N_FULL * VPP :].reshape(P * CV, R)
        parts.append(mid)
    parts.append(np.asarray(traw_arr).astype(np.float32))
    return np.concatenate(parts, axis=0)


def run(inputs: np.ndarray, **spmd_kwargs):
    inputs = np.ascontiguousarray(np.asarray(inputs, dtype=np.float32))
    assert inputs.shape == (N_VERTICES, R, F), inputs.shape
    shards = np.split(inputs, N_CORES, axis=0)
    in_maps = [{"inputs": np.ascontiguousarray(s)} for s in shards]
    res = run_bass_kernel_spmd(
        _get_nc(), in_maps, core_ids=list(range(N_CORES)), **spmd_kwargs
    )
    out = np.concatenate(
        [_decode_raw(r["raw"], r["traw"]) for r in res.results], axis=0
    )
    return out, res


def kernel(inputs: np.ndarray) -> np.ndarray:
    out, _ = run(inputs)
    return out


# revision 26
# speedup vs baseline: 1.0302x; 1.0302x over previous
"""Trainium2 Bass kernel for AngularMinPooling.

out[v, r] = inputs[v, r, argmin_j ||inputs[v, j, :]||_2]
Input (500000, 8, 64) f32 -> Output (500000, 8) f32.
Vertices are sharded across 8 NeuronCores; no cross-core communication.

HBM-bound (128 MB read per core). DMA tiles are 4 MB (VPP=16 vertices per
partition -> 32 KB contiguous per partition line) to amortize per-packet
SDMA dead time. Compute is chunked at 8 vertices/partition and split so no
engine exceeds the DMA floor:
  ACT    squares a chunk into sqd
  GpSimd folds sqd[...,0:32] + sqd[...,32:64] (halves the DVE reduce)
  DVE    reduces the fold to squared norms, min-reduces, builds the is_le
         one-hot, and does the one-hot weighted gather over the first 8
         feature columns (exact vs argmin except on bitwise-equal ties).
Output is staged in SBUF for the whole shard and written once,
partition-major (the host undoes the permutation).
"""

import os
import sys

import numpy as np

for _p in ("/opt/trn_rl_repo",):
    if os.path.isdir(_p) and _p not in sys.path:
        sys.path.insert(0, _p)

import concourse.bacc as bacc
import concourse.bass as bass
import concourse.tile as tile
from concourse import mybir
from concourse.bass_utils import run_bass_kernel_spmd


def _ensure_ntff_hook():
    """Install the axon NTFF profile hook if the image's antenv lacks it."""
    import types

    try:
        from antenv.axon_hooks import get_axon_ntff_profile_hook  # noqa: F401

        return
    except ImportError:
        pass
    try:
        import antenv
        from trn_agent_boot.trn_boot import _ntff_profile_via_ctypes

        mod = types.ModuleType("antenv.axon_hooks")
        _state = {"hook": None}
        mod.set_axon_ntff_profile_hook = lambda h: _state.__setitem__("hook", h)
        mod.get_axon_ntff_profile_hook = lambda: _state["hook"]
        sys.modules["antenv.axon_hooks"] = mod
        antenv.axon_hooks = mod
        so_path = "/opt/axon/libaxon_pjrt.so"
        if os.path.exists(so_path):
            mod.set_axon_ntff_profile_hook(_ntff_profile_via_ctypes(so_path))
    except Exception:
        pass


_ensure_ntff_hook()

N_VERTICES = 500_000
R = 8
F = 64
FH = F // 2  # folded feature width
N_CORES = 8
N_SHARD = N_VERTICES // N_CORES  # 62500 vertices per core
P = 128  # SBUF partitions
VPP = 16  # vertices per partition per DMA tile
CV = 8  # vertices per partition per compute chunk
TILE_V = P * VPP  # 2048 vertices per DMA tile
N_FULL = N_SHARD // TILE_V  # 30 full tiles (61440 vertices)
REM = N_SHARD - N_FULL * TILE_V  # 1060
N_MID = REM // (P * CV)  # 1 extra VPP=8 tile (1024 vertices)
TAIL = REM - N_MID * P * CV  # 36 leftover vertices
N_SLOTS = N_FULL * VPP + N_MID * CV  # 488 staged vertex slots per partition
KF = int(os.environ.get("AMP_KF", "0"))  # features folded in-place by GpSimd
SKEW = int(os.environ.get("AMP_SKEW", "0"))  # chunks of deferred gather emission
FLUSH_GP = os.environ.get("AMP_FLUSH", "sync") == "gp"  # stage-flush DMA engine

_DT = mybir.dt.float32
_AX = mybir.AxisListType
_OP = mybir.AluOpType


def _build_nc():
    nc = bacc.Bacc(
        "TRN2",
        target_bir_lowering=False,
        debug=False,
        enable_asserts=False,
        num_devices=N_CORES,
    )
    x = nc.dram_tensor("inputs", [N_SHARD, R, F], _DT, kind="ExternalInput")
    # Partition-major staged output; the host undoes the permutation.
    raw = nc.dram_tensor("raw", [P, N_SLOTS, R], _DT, kind="ExternalOutput")
    traw = nc.dram_tensor("traw", [TAIL, R], _DT, kind="ExternalOutput")
    xa = x.ap()

    with tile.TileContext(nc) as tc:
        with (
            tc.tile_pool(name="xin", bufs=4) as xin_pool,
            tc.tile_pool(name="sqd", bufs=2) as sqd_pool,
            tc.tile_pool(name="work", bufs=4) as work_pool,
            tc.tile_pool(name="stage", bufs=1) as stage_pool,
        ):
            stage = stage_pool.tile([P, N_SLOTS, R], _DT)

            # Software-pipelined emission: chunk c's front phase (square,
            # GP kfold, DVE reduce+min) is emitted immediately; its gather
            # phase (DVE is_le, GP mult+folds) is deferred until the next
            # chunk's front phase is in the queues. This keeps GpSimd's FIFO
            # free of head-of-line blocking: kfold(c+1) sits BEFORE
            # gather(c), so it never waits behind DVE's chunk-c tail.
            pending = []

            def front_chunk(xt_c, pc, vpp, ot_dst):
                """xt_c: [pc, vpp, R, F] slice of a loaded tile."""
                sqd = sqd_pool.tile([P, CV, R, F], _DT, tag="sqd")
                nc.scalar.square(sqd[:pc, :vpp], xt_c)

                # GpSimd folds the last KF squared features onto the previous
                # KF in place, so DVE only reduces F - KF features.
                if KF:
                    nc.gpsimd.tensor_tensor(
                        out=sqd[:pc, :vpp, :, F - 2 * KF : F - KF],
                        in0=sqd[:pc, :vpp, :, F - 2 * KF : F - KF],
                        in1=sqd[:pc, :vpp, :, F - KF : F],
                        op=_OP.add,
                    )
                sq = work_pool.tile([P, CV, R], _DT, tag="sq")
                nc.vector.tensor_reduce(
                    out=sq[:pc, :vpp],
                    in_=sqd[:pc, :vpp, :, 0 : F - KF],
                    axis=_AX.X,
                    op=_OP.add,
                )
                m = work_pool.tile([P, CV], _DT, tag="m")
                nc.vector.tensor_reduce(
                    out=m[:pc, :vpp], in_=sq[:pc, :vpp], axis=_AX.X, op=_OP.min
                )
                pending.append((xt_c, pc, vpp, sq, m, ot_dst))

            def gather_chunk(xt_c, pc, vpp, sq, m, ot_dst):
                # One-hot at the min norm (multi-hot only on bitwise-equal
                # ties, which have ~0 probability for random f32 sums).
                sel = work_pool.tile([P, CV, R], _DT, tag="sel")
                nc.vector.tensor_tensor(
                    out=sel[:pc, :vpp],
                    in0=sq[:pc, :vpp],
                    in1=m[:pc, :vpp, None].broadcast_to([pc, vpp, R]),
                    op=_OP.is_le,
                )
                # Gather via one-hot weighted sum over the first R feature
                # columns (argmin index is always < R); the sum over j is a
                # 3-level pairwise fold, all on GpSimd.
                g = work_pool.tile([P, CV, R, R], _DT, tag="g")
                nc.gpsimd.tensor_tensor(
                    out=g[:pc, :vpp],
                    in0=xt_c[:, :, :, 0:R],
                    in1=sel[:pc, :vpp, None, :].broadcast_to([pc, vpp, R, R]),
                    op=_OP.mult,
                )
                g2 = work_pool.tile([P, CV, R, R // 2], _DT, tag="g2")
                nc.gpsimd.tensor_tensor(
                    out=g2[:pc, :vpp],
                    in0=g[:pc, :vpp, :, 0 : R // 2],
                    in1=g[:pc, :vpp, :, R // 2 : R],
                    op=_OP.add,
                )
                g3 = work_pool.tile([P, CV, R, R // 4], _DT, tag="g3")
                nc.gpsimd.tensor_tensor(
                    out=g3[:pc, :vpp],
                    in0=g2[:pc, :vpp, :, 0 : R // 4],
                    in1=g2[:pc, :vpp, :, R // 4 : R // 2],
                    op=_OP.add,
                )
                nc.gpsimd.tensor_tensor(
                    out=ot_dst[:, :, :, None],
                    in0=g3[:pc, :vpp, :, 0:1],
                    in1=g3[:pc, :vpp, :, 1:2],
                    op=_OP.add,
                )

            def do_chunk(xt_c, pc, vpp, ot_dst, skew=SKEW):
                front_chunk(xt_c, pc, vpp, ot_dst)
                while len(pending) > skew:
                    gather_chunk(*pending.pop(0))

            def drain_chunks():
                while pending:
                    gather_chunk(*pending.pop(0))

            # Flush the staged output in slices as tiles complete so only a
            # small write remains after the last compute. (Keyed later when
            # skewed gather emission delays slice coverage.)
            _s = 1 if SKEW else 0
            flush_after = {
                7 + _s: (0, 128),
                15 + _s: (128, 256),
                23 + _s: (256, 384),
            }
            for t in range(N_FULL):
                v0 = t * TILE_V
                xt = xin_pool.tile([P, VPP, R, F], _DT, tag="xt")
                src = xa[v0 : v0 + TILE_V].rearrange("(p v) r f -> p v r f", p=P)
                dma_eng = nc.sync if t % 2 == 0 else nc.scalar
                dma_eng.dma_start(out=xt[:], in_=src)
                for c in range(VPP // CV):
                    s0 = t * VPP + c * CV
                    do_chunk(
                        xt[:, c * CV : (c + 1) * CV], P, CV,
                        stage[:, s0 : s0 + CV],
                    )
                if t in flush_after:
                    lo, hi = flush_after[t]
                    feng = nc.gpsimd if FLUSH_GP else nc.sync
                    feng.dma_start(out=raw.ap()[:, lo:hi], in_=stage[:, lo:hi])

            if N_MID:
                v0 = N_FULL * TILE_V
                xt = xin_pool.tile([P, VPP, R, F], _DT, tag="xt")
                src = xa[v0 : v0 + P * CV].rearrange("(p v) r f -> p v r f", p=P)
                nc.sync.dma_start(out=xt[:, :CV], in_=src)
                s0 = N_FULL * VPP
                do_chunk(xt[:, :CV], P, CV, stage[:, s0 : s0 + CV])

            if TAIL:
                v0 = N_FULL * TILE_V + N_MID * P * CV
                xt = xin_pool.tile([P, VPP, R, F], _DT, tag="xt")
                src = xa[v0 : v0 + TAIL].rearrange("(p v) r f -> p v r f", p=TAIL)
                nc.sync.dma_start(out=xt[:TAIL, :1], in_=src)
                ot_tail = work_pool.tile([P, 1, R], _DT, tag="ot_tail")
                do_chunk(xt[:TAIL, :1], TAIL, 1, ot_tail[:TAIL, :1])

            drain_chunks()
            feng = nc.gpsimd if FLUSH_GP else nc.sync
            feng.dma_start(out=raw.ap()[:, 384:], in_=stage[:, 384:])
            if TAIL:
                feng.dma_start(out=traw.ap(), in_=ot_tail[:TAIL, :1])
    nc.finalize()
    return nc


_NC_CACHE = None


def _get_nc():
    global _NC_CACHE
    if _NC_CACHE is None:
        _NC_CACHE = _build_nc()
    return _NC_CACHE


def _decode_raw(raw_arr: np.ndarray, traw_arr: np.ndarray) -> np.ndarray:
    """Map staged [P, N_SLOTS, R] output back to vertex order."""
    raw_arr = np.asarray(raw_arr).astype(np.float32)
    full = (
        raw_arr[:, : N_FULL * VPP]
        .reshape(P, N_FULL, VPP, R)
        .transpose(1, 0, 2, 3)
        .reshape(N_FULL * TILE_V, R)
    )
    parts = [full]
    if N_MID:
        mid = raw_arr[:, N_FULL * VPP :].reshape(P * CV, R)
        parts.append(mid)
    parts.append(np.asarray(traw_arr).astype(np.float32))
    return np.concatenate(parts, axis=0)


def run(inputs: np.ndarray, **spmd_kwargs):
    inputs = np.ascontiguousarray(np.asarray(inputs, dtype=np.float32))
    assert inputs.shape == (N_VERTICES, R, F), inputs.shape
    shards = np.split(inputs, N_CORES, axis=0)
    in_maps = [{"inputs": np.ascontiguousarray(s)} for s in shards]
    res = run_bass_kernel_spmd(
        _get_nc(), in_maps, core_ids=list(range(N_CORES)), **spmd_kwargs
    )
    out = np.concatenate(
        [_decode_raw(r["raw"], r["traw"]) for r in res.results], axis=0
    )
    return out, res


def kernel(inputs: np.ndarray) -> np.ndarray:
    out, _ = run(inputs)
    return out
